# revision 31
# baseline (speedup 1.0000x reference)
"""Trainium2 Bass kernel for nn_AutoregressiveHead.

Reference computation (L=32 heads, D=1024, H=512, B=8192, P=2 parents):
    base = einsum('bd,ldh->blh', x, Wx)
    parents = y[:, parent_idx]                  # [B, L, P]
    pc = einsum('blp,lph->blh', parents, Wp)
    h = relu(base + pc + b1)
    out = einsum('blh,lh->bl', h, W2) + b2      # [B, L]

Strategy (data-parallel over B across 8 NeuronCores, weights replicated):
  * Per core: B_core=1024 batch rows.  The dominant compute is the 32
    per-head matmuls x @ Wx[l] ([1024,1024]@[1024,512], bf16, fp32 PSUM).
  * The parent gather + b1 are folded into one extra K<=128 matmul:
    y_aug = [y^T; ones; 0-pad] (K=128 padded), W_aug[l] = [Wp scattered to
    dense label rows; b1[l]; 0] so  x@Wx[l] + y_aug@W_aug[l] = base+pc+b1.
  * |W2[l,h]| is folded into Wx/W_aug columns on the host, and columns are
    permuted so positive-sign-W2 columns come first.  Then
        out[b,l] = sum_pos relu(z') - sum_neg relu(z') + b2[l]
    which the Scalar engine computes with activation(Relu, accum_out=...)
    over the two column ranges -- no second matmul stage, h never stored.
  * PSUM tile [128, 512] per head; l processed in groups of 4 with the Wx
    slab for the group resident in SBUF (triple-buffered across groups).

Measured on HW (8 axon TRN2 cores): ~575 us exec (quiet terminal; the
interleaved equal-batch slope estimator in test.py; terminal load adds
up to ~20% session-level drift), rel err 0.0024.

Cost model (closed against measurement): every N=512 matmul "stream
pass" costs (512 + ~110)/2.4GHz = ~258 ns regardless of structure --
the ~110-cycle inter-pass bubble was invariant under: LDWEIGHTS dedup
(runs-of-4 same-lhsT with redundant LDWs deleted post-schedule, bit-
exact), identical rhs/lhsT operands, start=True non-accumulating mode,
and 2x/4x col-tiling via tile_position (all A/B'd on HW, bit-exact, 0
delta).  Main GEMM = 2048 passes * 258ns = 531us + y-block 37us +
ACT leak 9us ~= 577us.  So the kernel sits at this stack's floor;
only a pass-count reduction would beat it (fp8 DoubleRow halves passes
but both-operand e4m3 quantization measures rel err 0.038 > the 2e-2
gate; hybrid fractions that pass numerically save <5% -- rejected).

Tried and rejected (this + prior session A/B): fp8 DoubleRow
(numerics), K=64 row-split to 2 PSUM banks + DVE combine (this walrus
rejects any DVE instruction reading PSUM: birverifier assertion), DVE
tensor_scalar+accum (device crash), tensor_tensor_reduce ("ISA wrong
length"), G=8 runs-of-8 lhsT (-20%), PSUM-in-place relu (-4.5%),
y-strip LDW widening/dedup (bit-exact, 0 delta -- y LDWs already
hidden).  Post-schedule instruction-stream passes that MOVE
sem-carrying instructions deadlock the device (Tile wait counts are
baked against the scheduled order); delete-only/in-place-mutate passes
are safe.
"""

import os
import numpy as np
import ml_dtypes

import bass_rust
import concourse.bass as bass
import concourse.tile as tile
from concourse import mybir
from concourse.vector_clock import ScopedClock

BF16 = ml_dtypes.bfloat16

N_CORES = 8
B, D, H, L = 8192, 1024, 512, 32
B_CORE = B // N_CORES          # 1024
PPART = 128                    # partition size
KT = D // PPART                # 8 k-tiles over D
G = 4                          # heads per PSUM group
N_LG = L // G                  # 8 groups


class SplitDrainTileContext(tile.TileContext):
    """The walrus build in this container rejects >1 sem waits on the tail
    Drain ("Too many sync wait commands").  Redistribute the global-clock
    waits onto single-wait nops preceding the drain."""

    def _drain_and_barrier(self, tick_clock, wait_clock):
        probe = self.nc.sync.nop(nofuse=True)
        wait_clock.add_sem_waits(
            probe.ins, ScopedClock({None: tick_clock.global_clock})
        )
        si = probe.ins.sync_info
        waits = list(si.on_wait) if si is not None and si.on_wait else []
        if len(waits) > 1:
            si.on_wait = waits[:1]
            for w in waits[1:]:
                n = self.nc.sync.nop(nofuse=True)
                n.ins.sync_info = bass_rust.SyncInfo(on_wait=[w], on_update=[])
        self.nc.sync.drain()
        self.nc.all_engine_barrier()
        assert self.sems is not None
        popped = self.nc._tile_sem_poison_stack.pop()
        assert popped is self._sem_poison
        self.nc.clear_and_free_semaphores(list(self.sems.allocated().values()))
        self.nc.all_engine_barrier()


def split_multi_waits(nc, max_waits: int = 1):
    """This container's walrus rejects instructions carrying more than one
    sem-wait ("Too many sync wait commands").  Hoist extra waits onto
    single-wait NoOps inserted just before the instruction on the same
    engine (engine order is preserved; sems are monotonic, so waiting
    earlier on the same engine is equivalent)."""
    uid = 0
    for f in nc.m.functions:
        for blk in f.blocks:
            insts = blk.instructions
            new = []
            for inst in insts:
                si = inst.sync_info
                waits = list(si.on_wait) if si is not None and si.on_wait else []
                if len(waits) > max_waits:
                    for w in waits[:-max_waits]:
                        nop = mybir.InstNoOp(
                            name=f"splitw-{uid}", engine=inst.engine,
                            ins=[], outs=[],
                        )
                        uid += 1
                        nop.sync_info = bass_rust.SyncInfo(
                            on_wait=[w], on_update=[]
                        )
                        nc.register_instruction(nop, overwrite=True)
                        new.append(nop)
                    si.on_wait = waits[-max_waits:]
                new.append(inst)
            insts[:] = new


def reorder_dedup_pe(nc, mm_meta, n_kt=KT, n_g=G):
    """Rebuild the scheduled PE LDW/MM stream into k-outer/g-inner runs and
    drop the redundant per-MM LDWEIGHTS inside each same-lhsT run.

    The Tile scheduler emits one InstLdweights per InstMatmult (FIFO-paired,
    identical APs -- verified).  Within one (rep, lg, bt) iteration the four
    heads' matmuls at a given k-tile share lhsT = xT[k, bt]; loading it once
    and issuing the 4 matmuls back-to-back removes 3/4 of the weight-load
    traffic on the PE weight port, which is serialized with the rhs
    streaming on this hardware (measured 261 ns/mm at N=512 vs the 213 ns
    streaming floor).

    Per-PSUM-bank instruction order (start -> accumulate -> stop) is
    preserved: bank g still sees its k=0..7 matmuls in order followed by its
    y-strip matmul.  Sem waits are stall-only, so hoisting a dropped LDW's
    waits onto a nop placed just before the surviving LDW is equivalent.
    """
    import concourse.mybir as mybir_m

    blk = max(nc.m.functions[0].blocks, key=lambda b: len(b.instructions))
    insts = blk.instructions

    # FIFO-pair LDW -> MM on the PE stream
    from collections import deque
    q = deque()
    pairs = []      # (ldw, mm)
    for inst in insts:
        if getattr(inst, "engine", None) != mybir_m.EngineType.PE:
            continue
        if isinstance(inst, mybir_m.InstLdweights):
            q.append(inst)
        elif isinstance(inst, mybir_m.InstMatmult):
            pairs.append((q.popleft(), inst))
    assert not q, f"unpaired LDWEIGHTS: {len(q)}"

    groups = {}
    paired_ids = set()
    for ldw, mm in pairs:
        meta = mm_meta.get(mm.name)
        assert meta is not None, f"unknown matmul {mm.name}"
        rep, lg, bt, k, g, role = meta
        e = groups.setdefault((rep, lg, bt),
                              {"main": {}, "y": []})
        if role == "main":
            ent = e["main"].setdefault(k, {"ldws": [], "mms": {}})
            ent["ldws"].append(ldw)
            ent["mms"][g] = mm
        else:
            e["y"].append((g, ldw, mm))
        paired_ids.add(id(ldw))
        paired_ids.add(id(mm))

    uid = 0

    def wait_nops(waits):
        nonlocal uid
        out = []
        for w in waits:
            nop = mybir_m.InstNoOp(
                name=f"roww-{uid}", engine=mybir_m.EngineType.PE,
                ins=[], outs=[],
            )
            uid += 1
            nop.sync_info = bass_rust.SyncInfo(on_wait=[w], on_update=[])
            nc.register_instruction(nop, overwrite=True)
            out.append(nop)
        return out

    seq = []
    for key in sorted(groups):
        e = groups[key]
        for k in sorted(e["main"]):
            ent = e["main"][k]
            assert len(ent["ldws"]) == n_g and len(ent["mms"]) == n_g, (
                f"group {key} k={k}: {len(ent['ldws'])} ldws "
                f"{len(ent['mms'])} mms"
            )
            kept = ent["ldws"][0]
            extra_waits, extra_updates = [], []
            for d in ent["ldws"][1:]:
                si = d.sync_info
                if si is not None:
                    extra_waits += list(si.on_wait or [])
                    extra_updates += list(si.on_update or [])
            seq += wait_nops(extra_waits)
            if extra_updates:
                si = kept.sync_info
                if si is None:
                    kept.sync_info = bass_rust.SyncInfo(
                        on_wait=[], on_update=list(extra_updates))
                else:
                    si.on_update = list(si.on_update or []) + extra_updates
            seq.append(kept)
            for g in sorted(ent["mms"]):
                seq.append(ent["mms"][g])
        for g, ldw, mm in sorted(e["y"], key=lambda t: t[0]):
            seq += [ldw, mm]

    # splice: drop paired instructions, insert rebuilt sequence at the
    # position of the first dropped one
    new = []
    inserted = False
    for inst in insts:
        if id(inst) in paired_ids:
            if not inserted:
                new.extend(seq)
                inserted = True
            continue
        new.append(inst)
    assert inserted
    insts[:] = new


def ydedup_pe(nc, mm_meta):
    """Collapse each (lg, bt) group's four serial per-strip y-LDWEIGHTS
    (~107ns each on the PE weight port) into one 128-row load, by WIDENING
    the g0 strip LDW's access pattern in place and deleting the other three
    (they are sync-free, so deleting them changes no semaphore counts and
    no instruction moves).  The strip matmuls then contract against their
    32-row group of the one loaded array.

    A group is only rewritten when the scheduled stream is safe for it:
    g0's LDW precedes all four strip matmuls and no other LDWEIGHTS (which
    would clobber the array) sits between it and the last strip matmul.
    """
    import concourse.mybir as mybir_m
    from collections import deque

    blk = max(nc.m.functions[0].blocks, key=lambda b: len(b.instructions))
    insts = blk.instructions
    names = dict(mm_meta)

    strips = {}          # (rep, lg, bt) -> list[(g, ldw, mm)]
    q = deque()
    order = {}
    ldw_positions = []   # (pos, inst) of every LDW
    for pos, inst in enumerate(insts):
        if getattr(inst, "engine", None) != mybir_m.EngineType.PE:
            continue
        order[id(inst)] = pos
        if isinstance(inst, mybir_m.InstLdweights):
            q.append(inst)
            ldw_positions.append((pos, inst))
        elif isinstance(inst, mybir_m.InstMatmult):
            ldw = q.popleft()
            meta = names.get(inst.name)
            if meta is not None and meta[5] == "y":
                strips.setdefault(meta[:3], []).append((meta[4], ldw, inst))
    assert not q, f"unpaired LDWEIGHTS: {len(q)}"

    def sync_free(inst):
        si = inst.sync_info
        return si is None or (not si.on_wait and not si.on_update)

    drop = set()
    n_applied = n_skipped = 0
    for key, lst in strips.items():
        lst.sort(key=lambda t: t[0])
        if len(lst) != 4 or [g for g, _, _ in lst] != [0, 1, 2, 3]:
            n_skipped += 1
            continue
        ldw0 = lst[0][1]
        p0 = order[id(ldw0)]
        mm_pos = [order[id(mm)] for _, _, mm in lst]
        group_ldw_ids = {id(ldw) for _, ldw, _ in lst}
        if p0 >= min(mm_pos):
            n_skipped += 1
            continue
        if not all(sync_free(ldw) for _, ldw, _ in lst[1:]):
            n_skipped += 1
            continue
        # no foreign LDW may sit inside (p0, max(mm_pos))
        hi = max(mm_pos)
        foreign = any(p0 < pos < hi and id(ld) not in group_ldw_ids
                      for pos, ld in ldw_positions)
        if foreign:
            n_skipped += 1
            continue
        pap = ldw0.ins[0]
        ap = [list(d) for d in pap.ap]
        assert ap[0][1] == 32, f"unexpected strip LDW ap {ap}"
        ap[0][1] = 128
        pap.ap = ap
        ldw0.tile_size = (128, ldw0.tile_size[1])
        for _, ldw, _ in lst[1:]:
            drop.add(id(ldw))
        n_applied += 1

    if drop:
        insts[:] = [i for i in insts if id(i) not in drop]
    nc._ydedup_stats = (n_applied, n_skipped)


def build_program(n_bt: int, k_pos, repeats: int = 1, ytile: bool = True,
                  stage2: str = "act2", do_y: bool = True,
                  resident: bool = False, n_lg: int = N_LG,
                  reorder: bool = False, mm_mode: str = "normal",
                  colsplit: int = 1, rowsplit: bool = False,
                  onedma: bool = False):
    """Build the per-core Bass program.

    n_bt: number of 128-row batch tiles per core (8 for the real problem).
    k_pos: per-head split point -- columns [0:k_pos[l]) carry W2>=0,
           [k_pos[l]:H) carry W2<0 (after the host-side permutation).
    repeats: re-emit the whole compute `repeats` times (timing builds).
    ytile: pack the 4 per-head parent/bias matmuls of a group into
           concurrent 32-row-strip matmuls via tile_position (the strips
           execute in parallel in the PE array).
    """
    f32 = mybir.dt.float32
    bf16 = mybir.dt.bfloat16
    bc = n_bt * PPART

    nc = bass.Bass("TRN2", target_bir_lowering=False, debug=False,
                   num_devices=N_CORES)

    xT_d = nc.dram_tensor("xT", [D, bc], bf16, kind="ExternalInput")
    # wxp layout groups G heads contiguously per (lg, k) for 512KB DMAs
    wxp_d = nc.dram_tensor("wxp", [N_LG, KT, G, PPART, H], bf16,
                           kind="ExternalInput")
    b2r_d = nc.dram_tensor("b2r", [PPART, L], f32, kind="ExternalInput")
    if ytile:
        # per (lg): strip g holds [y_par0; y_par1; ones; 0...] for head lg*G+g
        yp_d = nc.dram_tensor("yp", [N_LG, PPART, bc], bf16,
                              kind="ExternalInput")
        wp_d = nc.dram_tensor("wp", [N_LG, PPART, H], bf16,
                              kind="ExternalInput")
    else:
        yp_d = nc.dram_tensor("ya", [PPART, bc], bf16, kind="ExternalInput")
        wp_d = nc.dram_tensor("wpa", [PPART, L, H], bf16,
                              kind="ExternalInput")
    out_d = nc.dram_tensor("out", [bc, L], f32, kind="ExternalOutput")

    with SplitDrainTileContext(nc) as tc:
        with (
            tc.tile_pool(name="const", bufs=1) as const_pool,
            tc.tile_pool(name="wx", bufs=3) as wx_pool,
            tc.tile_pool(name="psum", bufs=4 if rowsplit else 8,
                         space="PSUM") as psum_pool,
            tc.tile_pool(name="scratch", bufs=4) as scratch_pool,
            tc.tile_pool(name="outp", bufs=4) as out_pool,
        ):
            # --- resident tensors ---
            xt_sb = const_pool.tile([PPART, KT, bc], bf16, tag="xt")
            nc.sync.dma_start(
                xt_sb[:], xT_d.ap().rearrange("(kt p) b -> p kt b", p=PPART)
            )
            if ytile:
                yp_sb = const_pool.tile([PPART, N_LG, bc], bf16, tag="yp")
                nc.sync.dma_start(
                    yp_sb[:], yp_d.ap().rearrange("lg p b -> p lg b")
                )
                wp_sb = const_pool.tile([PPART, N_LG, H], bf16, tag="wp")
                nc.sync.dma_start(
                    wp_sb[:], wp_d.ap().rearrange("lg p h -> p lg h")
                )
            else:
                yp_sb = const_pool.tile([PPART, bc], bf16, tag="yp")
                nc.sync.dma_start(yp_sb[:], yp_d.ap())
                wp_sb = const_pool.tile([PPART, L, H], bf16, tag="wp")
                nc.sync.dma_start(wp_sb[:], wp_d.ap())
            b2_sb = const_pool.tile([PPART, L], f32, tag="b2")
            nc.sync.dma_start(b2_sb[:], b2r_d.ap())

            pos_sb = const_pool.tile([PPART, n_bt * L], f32, tag="pos")
            neg_sb = const_pool.tile([PPART, n_bt * L], f32, tag="neg")
            nc.vector.memset(pos_sb[:], 0.0)
            nc.vector.memset(neg_sb[:], 0.0)
            zero_sb = const_pool.tile([PPART, H], bf16, tag="zero")
            nc.vector.memset(zero_sb[:], 0.0)

            if resident:
                # diagnostic: all wx weights resident in SBUF (no DMA in the
                # repeat loop).  Only feasible for n_lg <= 4 (SBUF capacity).
                wxr_sb = const_pool.tile([PPART, n_lg, KT, G, H], bf16,
                                         tag="wxr")
                for lg in range(n_lg):
                    for k in range(KT):
                        nc.sync.dma_start(
                            wxr_sb[:, lg, k, :, :],
                            wxp_d.ap()[lg, k].rearrange("g p h -> p g h"),
                        )

            mm_meta = {}
            for _rep in range(repeats):
                for lg in range(n_lg):
                    if resident:
                        wx_sb = wxr_sb[:, lg]
                    elif onedma:
                        wx_sb = wx_pool.tile([PPART, KT, G, H], bf16, tag="wx")
                        nc.sync.dma_start(
                            wx_sb[:],
                            wxp_d.ap()[lg].rearrange("kt g p h -> p kt g h"),
                        )
                    else:
                        wx_sb = wx_pool.tile([PPART, KT, G, H], bf16, tag="wx")
                        for k in range(KT):
                            nc.sync.dma_start(
                                wx_sb[:, k, :, :],
                                wxp_d.ap()[lg, k].rearrange("g p h -> p g h"),
                            )
                    for bt in range(n_bt):
                        ps = [
                            psum_pool.tile([PPART, H], f32, tag="ps", name="ps")
                            for _ in range(G)
                        ]
                        if rowsplit:
                            # second bank per head: K rows 64..127 accumulate
                            # here; combined with ps[g] on DVE before stage 2.
                            psb = [
                                psum_pool.tile([PPART, H], f32, tag="psb",
                                               name="psb")
                                for _ in range(G)
                            ]
                        # g-outer / k-inner: lhsT changes every matmul, which
                        # measures ~1.5x faster per-mm than same-lhsT runs
                        # (PE weight-load pipelining quirk), and lets the
                        # ACT drain of ps[g] start before the bt finishes.
                        for g in range(G):
                            for k in range(KT):
                                lhsT = xt_sb[:, k,
                                             bt * PPART:(bt + 1) * PPART]
                                rhs = wx_sb[:, k, g, :]
                                start = (k == 0)
                                stop = (not do_y and k == KT - 1)
                                # mm_mode: timing diagnostics (results wrong
                                # except "normal")
                                if mm_mode in ("samerhs", "sameboth"):
                                    rhs = wx_sb[:, 0, 0, :]
                                if mm_mode in ("samelhs", "sameboth"):
                                    lhsT = xt_sb[:, 0, 0:PPART]
                                if mm_mode == "allstart":
                                    start = True
                                    stop = True
                                if rowsplit:
                                    # two K=64 strips to different banks:
                                    # streams run concurrently on disjoint
                                    # lane groups, drains to separate banks
                                    for r in range(2):
                                        tgt = ps[g] if r == 0 else psb[g]
                                        mi = nc.tensor.matmul(
                                            tgt[:],
                                            lhsT=xt_sb[64 * r:64 * (r + 1), k,
                                                       bt * PPART:
                                                       (bt + 1) * PPART],
                                            rhs=wx_sb[64 * r:64 * (r + 1),
                                                      k, g, :],
                                            start=start,
                                            stop=(r == 1 and k == KT - 1),
                                        )
                                        mm_meta[mi.ins.name] = (
                                            _rep, lg, bt, k, (g, r), "main")
                                elif colsplit == 1:
                                    mi = nc.tensor.matmul(
                                        ps[g][:], lhsT=lhsT, rhs=rhs,
                                        start=start, stop=stop,
                                        skip_group_check=(mm_mode == "allstart"),
                                    )
                                    mm_meta[mi.ins.name] = (_rep, lg, bt, k,
                                                            g, "main")
                                else:
                                    # col-tiled: M split into `colsplit`
                                    # strips at tile_position (0, 64c/32c);
                                    # strips stream concurrently on separate
                                    # xbuses, drains are partition-disjoint.
                                    mc = PPART // colsplit
                                    for c in range(colsplit):
                                        mi = nc.tensor.matmul(
                                            ps[g][mc * c:mc * (c + 1), :],
                                            lhsT=xt_sb[:, k,
                                                       bt * PPART + mc * c:
                                                       bt * PPART + mc * (c + 1)],
                                            rhs=rhs,
                                            start=start, stop=stop,
                                            tile_position=(0, mc * c),
                                        )
                                        mm_meta[mi.ins.name] = (
                                            _rep, lg, bt, k, (g, c), "main")
                        if not do_y:
                            pass
                        elif ytile:
                            for g in range(G):
                                mi = nc.tensor.matmul(
                                    ps[g][:],
                                    lhsT=yp_sb[32 * g:32 * (g + 1), lg,
                                               bt * PPART:(bt + 1) * PPART],
                                    rhs=wp_sb[32 * g:32 * (g + 1), lg, :],
                                    start=False, stop=True,
                                    tile_position=(32 * g, 0),
                                )
                                mm_meta[mi.ins.name] = (_rep, lg, bt, None, g,
                                                        "y")
                        else:
                            ya_lhs = yp_sb[:, bt * PPART:(bt + 1) * PPART]
                            for g in range(G):
                                nc.tensor.matmul(
                                    ps[g][:], lhsT=ya_lhs,
                                    rhs=wp_sb[:, lg * G + g, :],
                                    start=False, stop=True,
                                )
                        for g in range(G):
                            head = lg * G + g
                            kl = int(k_pos[head])
                            col = bt * L + head
                            if stage2 == "none":
                                continue
                            if rowsplit:
                                zt = scratch_pool.tile([PPART, H], f32,
                                                       tag="zt")
                                nc.vector.tensor_tensor(
                                    zt[:], ps[g][:], psb[g][:],
                                    mybir.AluOpType.add,
                                )
                                sc = scratch_pool.tile([PPART, H], bf16,
                                                       tag="sc")
                                if kl > 0:
                                    nc.scalar.activation(
                                        sc[:, :kl], zt[:, :kl],
                                        mybir.ActivationFunctionType.Relu,
                                        accum_out=pos_sb[:, col:col + 1],
                                    )
                                if kl < H:
                                    nc.scalar.activation(
                                        sc[:, kl:], zt[:, kl:],
                                        mybir.ActivationFunctionType.Relu,
                                        accum_out=neg_sb[:, col:col + 1],
                                    )
                                continue
                            if stage2 == "act2p":
                                # relu main-out written back to PSUM in
                                # place: faster ACT access path and no
                                # SBUF scratch writes contending with the
                                # PE's rhs streams
                                if kl > 0:
                                    nc.scalar.activation(
                                        ps[g][:, :kl], ps[g][:, :kl],
                                        mybir.ActivationFunctionType.Relu,
                                        accum_out=pos_sb[:, col:col + 1],
                                    )
                                if kl < H:
                                    nc.scalar.activation(
                                        ps[g][:, kl:], ps[g][:, kl:],
                                        mybir.ActivationFunctionType.Relu,
                                        accum_out=neg_sb[:, col:col + 1],
                                    )
                                continue
                            sc = scratch_pool.tile([PPART, H], bf16, tag="sc")
                            if kl > 0:
                                nc.scalar.activation(
                                    sc[:, :kl], ps[g][:, :kl],
                                    mybir.ActivationFunctionType.Relu,
                                    accum_out=pos_sb[:, col:col + 1],
                                )
                            if kl < H:
                                if stage2 == "ttr":
                                    # negative-sign range reduced on DVE
                                    # (tensor_tensor_reduce, as in the QR
                                    # kernel) to halve the ScalarE load:
                                    # accum = sum(max(z, 0))
                                    scd = scratch_pool.tile(
                                        [PPART, H], bf16, tag="scd")
                                    nc.vector.tensor_tensor_reduce(
                                        scd[:, kl:], ps[g][:, kl:],
                                        zero_sb[:, :H - kl], 1.0, 0.0,
                                        mybir.AluOpType.max,
                                        mybir.AluOpType.add,
                                        accum_out=neg_sb[:, col:col + 1],
                                    )
                                elif stage2 == "split":
                                    # negative-sign range reduced on DVE to
                                    # halve the ScalarE load
                                    nc.vector.tensor_scalar(
                                        sc[:, kl:], ps[g][:, kl:],
                                        0.0, None, mybir.AluOpType.max,
                                        op1=mybir.AluOpType.add,
                                        accum_out=neg_sb[:, col:col + 1],
                                    )
                                else:
                                    nc.scalar.activation(
                                        sc[:, kl:], ps[g][:, kl:],
                                        mybir.ActivationFunctionType.Relu,
                                        accum_out=neg_sb[:, col:col + 1],
                                    )

            # --- epilogue: out = pos - neg + b2 ---
            for bt in range(n_bt):
                o = out_pool.tile([PPART, L], f32, tag="o")
                nc.vector.tensor_tensor(
                    o[:], pos_sb[:, bt * L:(bt + 1) * L],
                    neg_sb[:, bt * L:(bt + 1) * L], mybir.AluOpType.subtract,
                )
                nc.vector.tensor_tensor(
                    o[:], o[:], b2_sb[:], mybir.AluOpType.add,
                )
                nc.sync.dma_start(out_d.ap()[bt * PPART:(bt + 1) * PPART, :], o[:])

    if rowsplit:
        assert ytile and do_y and colsplit == 1 and mm_mode == "normal"
    if reorder == "y":
        assert ytile and do_y
        ydedup_pe(nc, mm_meta)
    elif reorder:
        assert ytile and do_y and not resident
        reorder_dedup_pe(nc, mm_meta)
    split_multi_waits(nc)
    return nc


def prep_host(x, y, Wx, Wp, b1, W2, b2, parent_idx, n_bt: int = 8,
              ytile: bool = True):
    """Host-side data prep.  Returns (in_maps per core, k_pos)."""
    x = np.asarray(x, np.float32)
    y = np.asarray(y, np.float32)
    Wx = np.asarray(Wx, np.float32)
    Wp = np.asarray(Wp, np.float32)
    b1 = np.asarray(b1, np.float32)
    W2 = np.asarray(W2, np.float32)
    b2 = np.asarray(b2, np.float32)
    parent_idx = np.asarray(parent_idx)
    NP = parent_idx.shape[1]

    bc = n_bt * PPART

    # |W2| folding + sign-partition permutation of the H axis (per head)
    s = np.abs(W2)                       # [L, H]
    k_pos = np.zeros(L, np.int64)
    perm = np.zeros((L, H), np.int64)
    for l in range(L):
        posm = W2[l] >= 0
        p_idx = np.concatenate([np.where(posm)[0], np.where(~posm)[0]])
        perm[l] = p_idx
        k_pos[l] = int(posm.sum())

    # wxp[lg, kt, g, p, h'] = Wx[l, kt*128+p, perm[l,h']] * s[l, perm[l,h']]
    wxp = np.empty((N_LG, KT, G, PPART, H), BF16)
    for l in range(L):
        m = (Wx[l] * s[l][None, :])[:, perm[l]]          # [D, H]
        wxp[l // G, :, l % G] = m.reshape(KT, PPART, H).astype(BF16)

    b2r = np.broadcast_to(b2[None, :], (PPART, L)).astype(np.float32).copy()

    if ytile:
        # wp[lg, 32g+j] = parent-j weights of head l=lg*G+g (scaled/permuted);
        # row 32g+NP = b1 row (pairs with the ones row of yp).
        wp = np.zeros((N_LG, PPART, H), np.float32)
        for l in range(L):
            lg, g = l // G, l % G
            for j in range(NP):
                wp[lg, 32 * g + j] = (Wp[l, j] * s[l])[perm[l]]
            wp[lg, 32 * g + NP] = (b1[l] * s[l])[perm[l]]
        wp = wp.astype(BF16)
    else:
        # dense label-indexed parent weights + b1 via ones row (K=128 padded)
        Wfull = np.zeros((L, L, H), np.float32)
        for l in range(L):
            for p in range(NP):
                Wfull[l, parent_idx[l, p]] += Wp[l, p]
        wpa = np.zeros((PPART, L, H), np.float32)
        for l in range(L):
            wpa[:L, l, :] = (Wfull[l] * s[l][None, :])[:, perm[l]]
            wpa[L, l, :] = (b1[l] * s[l])[perm[l]]
        wpa = wpa.astype(BF16)

    in_maps = []
    for c in range(N_CORES):
        xs = x[c * bc:(c + 1) * bc]                       # [bc, D]
        ys = y[c * bc:(c + 1) * bc]                       # [bc, L]
        xT = np.ascontiguousarray(xs.T).astype(BF16)      # [D, bc]
        m = {"xT": xT, "wxp": wxp, "b2r": b2r}
        if ytile:
            yp = np.zeros((N_LG, PPART, bc), np.float32)
            for l in range(L):
                lg, g = l // G, l % G
                for j in range(NP):
                    yp[lg, 32 * g + j] = ys[:, parent_idx[l, j]]
                yp[lg, 32 * g + NP] = 1.0
            m["yp"] = yp.astype(BF16)
            m["wp"] = wp
        else:
            ya = np.zeros((PPART, bc), np.float32)
            ya[:L] = ys.T
            ya[L] = 1.0
            m["ya"] = ya.astype(BF16)
            m["wpa"] = wpa
        in_maps.append(m)
    return in_maps, k_pos


def reference_host(x, y, Wx, Wp, b1, W2, b2, parent_idx):
    """numpy fp32 reference (for sim tests inside this module)."""
    base = np.einsum('bd,ldh->blh', x, Wx)
    parents = y[:, parent_idx]
    pc = np.einsum('blp,lph->blh', parents, Wp)
    h = np.maximum(base + pc + b1[None], 0.0)
    return np.einsum('blh,lh->bl', h, W2) + b2


_CACHE = {}


def kernel(x, y, Wx, Wp, b1, W2, b2, parent_idx):
    from concourse.bass_utils import run_bass_kernel_spmd

    x = np.asarray(x)
    n_bt = x.shape[0] // N_CORES // PPART
    in_maps, k_pos = prep_host(x, y, Wx, Wp, b1, W2, b2, parent_idx, n_bt=n_bt)

    key = (n_bt, True, tuple(int(v) for v in k_pos))
    if key not in _CACHE:
        _CACHE[key] = build_program(n_bt, k_pos)
    nc = _CACHE[key]

    res = run_bass_kernel_spmd(nc, in_maps, core_ids=list(range(N_CORES)))
    out = np.concatenate([res.results[c]["out"] for c in range(N_CORES)], axis=0)
    return out.astype(np.float32)



# revision 33
# speedup vs baseline: 1.0510x; 1.0510x over previous
"""Trainium2 Bass kernel for nn_AutoregressiveHead.

Reference computation (L=32 heads, D=1024, H=512, B=8192, P=2 parents):
    base = einsum('bd,ldh->blh', x, Wx)
    parents = y[:, parent_idx]                  # [B, L, P]
    pc = einsum('blp,lph->blh', parents, Wp)
    h = relu(base + pc + b1)
    out = einsum('blh,lh->bl', h, W2) + b2      # [B, L]

Strategy (data-parallel over B across 8 NeuronCores, weights replicated):
  * Per core: B_core=1024 batch rows.  The dominant compute is the 32
    per-head matmuls x @ Wx[l] ([1024,1024]@[1024,512], bf16, fp32 PSUM).
  * The parent gather + b1 are folded into one extra K<=128 matmul:
    y_aug = [y^T; ones; 0-pad] (K=128 padded), W_aug[l] = [Wp scattered to
    dense label rows; b1[l]; 0] so  x@Wx[l] + y_aug@W_aug[l] = base+pc+b1.
  * |W2[l,h]| is folded into Wx/W_aug columns on the host, and columns are
    permuted so positive-sign-W2 columns come first.  Then
        out[b,l] = sum_pos relu(z') - sum_neg relu(z') + b2[l]
    which the Scalar engine computes with activation(Relu, accum_out=...)
    over the two column ranges -- no second matmul stage, h never stored.
  * PSUM tile [128, 512] per head; l processed in groups of 4 with the Wx
    slab for the group resident in SBUF (triple-buffered across groups).

Measured on HW (8 axon TRN2 cores): ~575 us exec (quiet terminal; the
interleaved equal-batch slope estimator in test.py; terminal load adds
up to ~20% session-level drift), rel err 0.0024.

Cost model (closed against measurement): every N=512 matmul "stream
pass" costs ~258 ns = 512 columns at an effective ~2.0 GHz (the P0
sustained-load downclock), regardless of structure.  Invariant under:
LDWEIGHTS dedup (runs-of-4 same-lhsT with redundant LDWs deleted
post-schedule, bit-exact), identical rhs/lhsT operands, start=True
non-accumulating mode, 2x/4x col-tiling via tile_position, single-DMA
weight slabs, and -- decisively -- an rhs bitcast to fp8 (same column
count, HALF the bytes: identical 572.9us vs 572.9us base), which rules
out per-xbus byte rate and pins the wall to column-streaming rate
alone.  Main GEMM = 2048 passes * 258ns = 531us + y-block 37us + ACT
leak 9us ~= 577us.  Only a pass-count reduction can beat this, and the
only mechanism (fp8 DoubleRow, K=256/pass, needs BOTH operands e4m3)
measures rel err 0.038 > the 2e-2 gate; passing hybrid fractions save
<5% while eating the correctness margin -- rejected.

Tried and rejected (this + prior session A/B): fp8 DoubleRow
(numerics), K=64 row-split to 2 PSUM banks + DVE combine (this walrus
rejects any DVE instruction reading PSUM: birverifier assertion), DVE
tensor_scalar+accum (device crash), tensor_tensor_reduce ("ISA wrong
length"), G=8 runs-of-8 lhsT (-20%), PSUM-in-place relu (-4.5%),
y-strip LDW widening/dedup (bit-exact, 0 delta -- y LDWs already
hidden).  Post-schedule instruction-stream passes that MOVE
sem-carrying instructions deadlock the device (Tile wait counts are
baked against the scheduled order); delete-only/in-place-mutate passes
are safe.
"""

import os
import numpy as np
import ml_dtypes

import bass_rust
import concourse.bass as bass
import concourse.tile as tile
from concourse import mybir
from concourse.vector_clock import ScopedClock

BF16 = ml_dtypes.bfloat16

N_CORES = 8
B, D, H, L = 8192, 1024, 512, 32
B_CORE = B // N_CORES          # 1024
PPART = 128                    # partition size
KT = D // PPART                # 8 k-tiles over D
G = 4                          # heads per PSUM group
N_LG = L // G                  # 8 groups


class SplitDrainTileContext(tile.TileContext):
    """The walrus build in this container rejects >1 sem waits on the tail
    Drain ("Too many sync wait commands").  Redistribute the global-clock
    waits onto single-wait nops preceding the drain."""

    def _drain_and_barrier(self, tick_clock, wait_clock):
        probe = self.nc.sync.nop(nofuse=True)
        wait_clock.add_sem_waits(
            probe.ins, ScopedClock({None: tick_clock.global_clock})
        )
        si = probe.ins.sync_info
        waits = list(si.on_wait) if si is not None and si.on_wait else []
        if len(waits) > 1:
            si.on_wait = waits[:1]
            for w in waits[1:]:
                n = self.nc.sync.nop(nofuse=True)
                n.ins.sync_info = bass_rust.SyncInfo(on_wait=[w], on_update=[])
        self.nc.sync.drain()
        self.nc.all_engine_barrier()
        assert self.sems is not None
        popped = self.nc._tile_sem_poison_stack.pop()
        assert popped is self._sem_poison
        self.nc.clear_and_free_semaphores(list(self.sems.allocated().values()))
        self.nc.all_engine_barrier()


def split_multi_waits(nc, max_waits: int = 1):
    """This container's walrus rejects instructions carrying more than one
    sem-wait ("Too many sync wait commands").  Hoist extra waits onto
    single-wait NoOps inserted just before the instruction on the same
    engine (engine order is preserved; sems are monotonic, so waiting
    earlier on the same engine is equivalent)."""
    uid = 0
    for f in nc.m.functions:
        for blk in f.blocks:
            insts = blk.instructions
            new = []
            for inst in insts:
                si = inst.sync_info
                waits = list(si.on_wait) if si is not None and si.on_wait else []
                if len(waits) > max_waits:
                    for w in waits[:-max_waits]:
                        nop = mybir.InstNoOp(
                            name=f"splitw-{uid}", engine=inst.engine,
                            ins=[], outs=[],
                        )
                        uid += 1
                        nop.sync_info = bass_rust.SyncInfo(
                            on_wait=[w], on_update=[]
                        )
                        nc.register_instruction(nop, overwrite=True)
                        new.append(nop)
                    si.on_wait = waits[-max_waits:]
                new.append(inst)
            insts[:] = new


def reorder_dedup_pe(nc, mm_meta, n_kt=KT, n_g=G):
    """Rebuild the scheduled PE LDW/MM stream into k-outer/g-inner runs and
    drop the redundant per-MM LDWEIGHTS inside each same-lhsT run.

    The Tile scheduler emits one InstLdweights per InstMatmult (FIFO-paired,
    identical APs -- verified).  Within one (rep, lg, bt) iteration the four
    heads' matmuls at a given k-tile share lhsT = xT[k, bt]; loading it once
    and issuing the 4 matmuls back-to-back removes 3/4 of the weight-load
    traffic on the PE weight port, which is serialized with the rhs
    streaming on this hardware (measured 261 ns/mm at N=512 vs the 213 ns
    streaming floor).

    Per-PSUM-bank instruction order (start -> accumulate -> stop) is
    preserved: bank g still sees its k=0..7 matmuls in order followed by its
    y-strip matmul.  Sem waits are stall-only, so hoisting a dropped LDW's
    waits onto a nop placed just before the surviving LDW is equivalent.
    """
    import concourse.mybir as mybir_m

    blk = max(nc.m.functions[0].blocks, key=lambda b: len(b.instructions))
    insts = blk.instructions

    # FIFO-pair LDW -> MM on the PE stream
    from collections import deque
    q = deque()
    pairs = []      # (ldw, mm)
    for inst in insts:
        if getattr(inst, "engine", None) != mybir_m.EngineType.PE:
            continue
        if isinstance(inst, mybir_m.InstLdweights):
            q.append(inst)
        elif isinstance(inst, mybir_m.InstMatmult):
            pairs.append((q.popleft(), inst))
    assert not q, f"unpaired LDWEIGHTS: {len(q)}"

    groups = {}
    paired_ids = set()
    for ldw, mm in pairs:
        meta = mm_meta.get(mm.name)
        assert meta is not None, f"unknown matmul {mm.name}"
        rep, lg, bt, k, g, role = meta
        e = groups.setdefault((rep, lg, bt),
                              {"main": {}, "y": []})
        if role == "main":
            ent = e["main"].setdefault(k, {"ldws": [], "mms": {}})
            ent["ldws"].append(ldw)
            ent["mms"][g] = mm
        else:
            e["y"].append((g, ldw, mm))
        paired_ids.add(id(ldw))
        paired_ids.add(id(mm))

    uid = 0

    def wait_nops(waits):
        nonlocal uid
        out = []
        for w in waits:
            nop = mybir_m.InstNoOp(
                name=f"roww-{uid}", engine=mybir_m.EngineType.PE,
                ins=[], outs=[],
            )
            uid += 1
            nop.sync_info = bass_rust.SyncInfo(on_wait=[w], on_update=[])
            nc.register_instruction(nop, overwrite=True)
            out.append(nop)
        return out

    seq = []
    for key in sorted(groups):
        e = groups[key]
        for k in sorted(e["main"]):
            ent = e["main"][k]
            assert len(ent["ldws"]) == n_g and len(ent["mms"]) == n_g, (
                f"group {key} k={k}: {len(ent['ldws'])} ldws "
                f"{len(ent['mms'])} mms"
            )
            kept = ent["ldws"][0]
            extra_waits, extra_updates = [], []
            for d in ent["ldws"][1:]:
                si = d.sync_info
                if si is not None:
                    extra_waits += list(si.on_wait or [])
                    extra_updates += list(si.on_update or [])
            seq += wait_nops(extra_waits)
            if extra_updates:
                si = kept.sync_info
                if si is None:
                    kept.sync_info = bass_rust.SyncInfo(
                        on_wait=[], on_update=list(extra_updates))
                else:
                    si.on_update = list(si.on_update or []) + extra_updates
            seq.append(kept)
            for g in sorted(ent["mms"]):
                seq.append(ent["mms"][g])
        for g, ldw, mm in sorted(e["y"], key=lambda t: t[0]):
            seq += [ldw, mm]

    # splice: drop paired instructions, insert rebuilt sequence at the
    # position of the first dropped one
    new = []
    inserted = False
    for inst in insts:
        if id(inst) in paired_ids:
            if not inserted:
                new.extend(seq)
                inserted = True
            continue
        new.append(inst)
    assert inserted
    insts[:] = new


def ydedup_pe(nc, mm_meta):
    """Collapse each (lg, bt) group's four serial per-strip y-LDWEIGHTS
    (~107ns each on the PE weight port) into one 128-row load, by WIDENING
    the g0 strip LDW's access pattern in place and deleting the other three
    (they are sync-free, so deleting them changes no semaphore counts and
    no instruction moves).  The strip matmuls then contract against their
    32-row group of the one loaded array.

    A group is only rewritten when the scheduled stream is safe for it:
    g0's LDW precedes all four strip matmuls and no other LDWEIGHTS (which
    would clobber the array) sits between it and the last strip matmul.
    """
    import concourse.mybir as mybir_m
    from collections import deque

    blk = max(nc.m.functions[0].blocks, key=lambda b: len(b.instructions))
    insts = blk.instructions
    names = dict(mm_meta)

    strips = {}          # (rep, lg, bt) -> list[(g, ldw, mm)]
    q = deque()
    order = {}
    ldw_positions = []   # (pos, inst) of every LDW
    for pos, inst in enumerate(insts):
        if getattr(inst, "engine", None) != mybir_m.EngineType.PE:
            continue
        order[id(inst)] = pos
        if isinstance(inst, mybir_m.InstLdweights):
            q.append(inst)
            ldw_positions.append((pos, inst))
        elif isinstance(inst, mybir_m.InstMatmult):
            ldw = q.popleft()
            meta = names.get(inst.name)
            if meta is not None and meta[5] == "y":
                strips.setdefault(meta[:3], []).append((meta[4], ldw, inst))
    assert not q, f"unpaired LDWEIGHTS: {len(q)}"

    def sync_free(inst):
        si = inst.sync_info
        return si is None or (not si.on_wait and not si.on_update)

    drop = set()
    n_applied = n_skipped = 0
    for key, lst in strips.items():
        lst.sort(key=lambda t: t[0])
        if len(lst) != 4 or [g for g, _, _ in lst] != [0, 1, 2, 3]:
            n_skipped += 1
            continue
        ldw0 = lst[0][1]
        p0 = order[id(ldw0)]
        mm_pos = [order[id(mm)] for _, _, mm in lst]
        group_ldw_ids = {id(ldw) for _, ldw, _ in lst}
        if p0 >= min(mm_pos):
            n_skipped += 1
            continue
        if not all(sync_free(ldw) for _, ldw, _ in lst[1:]):
            n_skipped += 1
            continue
        # no foreign LDW may sit inside (p0, max(mm_pos))
        hi = max(mm_pos)
        foreign = any(p0 < pos < hi and id(ld) not in group_ldw_ids
                      for pos, ld in ldw_positions)
        if foreign:
            n_skipped += 1
            continue
        pap = ldw0.ins[0]
        ap = [list(d) for d in pap.ap]
        assert ap[0][1] == 32, f"unexpected strip LDW ap {ap}"
        ap[0][1] = 128
        pap.ap = ap
        ldw0.tile_size = (128, ldw0.tile_size[1])
        for _, ldw, _ in lst[1:]:
            drop.add(id(ldw))
        n_applied += 1

    if drop:
        insts[:] = [i for i in insts if id(i) not in drop]
    nc._ydedup_stats = (n_applied, n_skipped)


def build_program(n_bt: int, k_pos, repeats: int = 1, ytile: bool = True,
                  stage2: str = "act2", do_y: bool = True,
                  resident: bool = False, n_lg: int = N_LG,
                  reorder: bool = False, mm_mode: str = "normal",
                  colsplit: int = 1, rowsplit: bool = False,
                  onedma: bool = False):
    """Build the per-core Bass program.

    n_bt: number of 128-row batch tiles per core (8 for the real problem).
    k_pos: per-head split point -- columns [0:k_pos[l]) carry W2>=0,
           [k_pos[l]:H) carry W2<0 (after the host-side permutation).
    repeats: re-emit the whole compute `repeats` times (timing builds).
    ytile: pack the 4 per-head parent/bias matmuls of a group into
           concurrent 32-row-strip matmuls via tile_position (the strips
           execute in parallel in the PE array).
    """
    f32 = mybir.dt.float32
    bf16 = mybir.dt.bfloat16
    bc = n_bt * PPART

    nc = bass.Bass("TRN2", target_bir_lowering=False, debug=False,
                   num_devices=N_CORES)

    xT_d = nc.dram_tensor("xT", [D, bc], bf16, kind="ExternalInput")
    # wxp layout groups G heads contiguously per (lg, k) for 512KB DMAs
    wxp_d = nc.dram_tensor("wxp", [N_LG, KT, G, PPART, H], bf16,
                           kind="ExternalInput")
    b2r_d = nc.dram_tensor("b2r", [PPART, L], f32, kind="ExternalInput")
    if ytile:
        # per (lg): strip g holds [y_par0; y_par1; ones; 0...] for head lg*G+g
        yp_d = nc.dram_tensor("yp", [N_LG, PPART, bc], bf16,
                              kind="ExternalInput")
        wp_d = nc.dram_tensor("wp", [N_LG, PPART, H], bf16,
                              kind="ExternalInput")
    else:
        yp_d = nc.dram_tensor("ya", [PPART, bc], bf16, kind="ExternalInput")
        wp_d = nc.dram_tensor("wpa", [PPART, L, H], bf16,
                              kind="ExternalInput")
    out_d = nc.dram_tensor("out", [bc, L], f32, kind="ExternalOutput")

    with SplitDrainTileContext(nc) as tc:
        with (
            tc.tile_pool(name="const", bufs=1) as const_pool,
            tc.tile_pool(name="wx", bufs=3) as wx_pool,
            tc.tile_pool(name="psum", bufs=4 if rowsplit else 8,
                         space="PSUM") as psum_pool,
            tc.tile_pool(name="scratch", bufs=4) as scratch_pool,
            tc.tile_pool(name="outp", bufs=4) as out_pool,
        ):
            # --- resident tensors ---
            xt_sb = const_pool.tile([PPART, KT, bc], bf16, tag="xt")
            nc.sync.dma_start(
                xt_sb[:], xT_d.ap().rearrange("(kt p) b -> p kt b", p=PPART)
            )
            if ytile:
                yp_sb = const_pool.tile([PPART, N_LG, bc], bf16, tag="yp")
                nc.sync.dma_start(
                    yp_sb[:], yp_d.ap().rearrange("lg p b -> p lg b")
                )
                wp_sb = const_pool.tile([PPART, N_LG, H], bf16, tag="wp")
                nc.sync.dma_start(
                    wp_sb[:], wp_d.ap().rearrange("lg p h -> p lg h")
                )
            else:
                yp_sb = const_pool.tile([PPART, bc], bf16, tag="yp")
                nc.sync.dma_start(yp_sb[:], yp_d.ap())
                wp_sb = const_pool.tile([PPART, L, H], bf16, tag="wp")
                nc.sync.dma_start(wp_sb[:], wp_d.ap())
            b2_sb = const_pool.tile([PPART, L], f32, tag="b2")
            nc.sync.dma_start(b2_sb[:], b2r_d.ap())

            pos_sb = const_pool.tile([PPART, n_bt * L], f32, tag="pos")
            neg_sb = const_pool.tile([PPART, n_bt * L], f32, tag="neg")
            nc.vector.memset(pos_sb[:], 0.0)
            nc.vector.memset(neg_sb[:], 0.0)
            zero_sb = const_pool.tile([PPART, H], bf16, tag="zero")
            nc.vector.memset(zero_sb[:], 0.0)

            if resident:
                # diagnostic: all wx weights resident in SBUF (no DMA in the
                # repeat loop).  Only feasible for n_lg <= 4 (SBUF capacity).
                wxr_sb = const_pool.tile([PPART, n_lg, KT, G, H], bf16,
                                         tag="wxr")
                for lg in range(n_lg):
                    for k in range(KT):
                        nc.sync.dma_start(
                            wxr_sb[:, lg, k, :, :],
                            wxp_d.ap()[lg, k].rearrange("g p h -> p g h"),
                        )

            mm_meta = {}
            for _rep in range(repeats):
                for lg in range(n_lg):
                    if resident:
                        wx_sb = wxr_sb[:, lg]
                    elif onedma:
                        wx_sb = wx_pool.tile([PPART, KT, G, H], bf16, tag="wx")
                        nc.sync.dma_start(
                            wx_sb[:],
                            wxp_d.ap()[lg].rearrange("kt g p h -> p kt g h"),
                        )
                    else:
                        wx_sb = wx_pool.tile([PPART, KT, G, H], bf16, tag="wx")
                        for k in range(KT):
                            nc.sync.dma_start(
                                wx_sb[:, k, :, :],
                                wxp_d.ap()[lg, k].rearrange("g p h -> p g h"),
                            )
                    for bt in range(n_bt):
                        ps = [
                            psum_pool.tile([PPART, H], f32, tag="ps", name="ps")
                            for _ in range(G)
                        ]
                        if rowsplit:
                            # second bank per head: K rows 64..127 accumulate
                            # here; combined with ps[g] on DVE before stage 2.
                            psb = [
                                psum_pool.tile([PPART, H], f32, tag="psb",
                                               name="psb")
                                for _ in range(G)
                            ]
                        # g-outer / k-inner: lhsT changes every matmul, which
                        # measures ~1.5x faster per-mm than same-lhsT runs
                        # (PE weight-load pipelining quirk), and lets the
                        # ACT drain of ps[g] start before the bt finishes.
                        for g in range(G):
                            for k in range(KT):
                                lhsT = xt_sb[:, k,
                                             bt * PPART:(bt + 1) * PPART]
                                rhs = wx_sb[:, k, g, :]
                                start = (k == 0)
                                stop = (not do_y and k == KT - 1)
                                # mm_mode: timing diagnostics (results wrong
                                # except "normal")
                                if mm_mode in ("samerhs", "sameboth"):
                                    rhs = wx_sb[:, 0, 0, :]
                                if mm_mode in ("samelhs", "sameboth"):
                                    lhsT = xt_sb[:, 0, 0:PPART]
                                if mm_mode == "rhs8":
                                    # timing probe: same column count, half
                                    # the rhs bytes (values are garbage)
                                    rhs = wx_sb[:, k, g, :].bitcast(
                                        mybir.dt.float8e4)[:, :H]
                                if mm_mode == "allstart":
                                    start = True
                                    stop = True
                                if rowsplit:
                                    # two K=64 strips to different banks:
                                    # streams run concurrently on disjoint
                                    # lane groups, drains to separate banks
                                    for r in range(2):
                                        tgt = ps[g] if r == 0 else psb[g]
                                        mi = nc.tensor.matmul(
                                            tgt[:],
                                            lhsT=xt_sb[64 * r:64 * (r + 1), k,
                                                       bt * PPART:
                                                       (bt + 1) * PPART],
                                            rhs=wx_sb[64 * r:64 * (r + 1),
                                                      k, g, :],
                                            start=start,
                                            stop=(r == 1 and k == KT - 1),
                                        )
                                        mm_meta[mi.ins.name] = (
                                            _rep, lg, bt, k, (g, r), "main")
                                elif colsplit == 1:
                                    mi = nc.tensor.matmul(
                                        ps[g][:], lhsT=lhsT, rhs=rhs,
                                        start=start, stop=stop,
                                        skip_group_check=(mm_mode == "allstart"),
                                    )
                                    mm_meta[mi.ins.name] = (_rep, lg, bt, k,
                                                            g, "main")
                                else:
                                    # col-tiled: M split into `colsplit`
                                    # strips at tile_position (0, 64c/32c);
                                    # strips stream concurrently on separate
                                    # xbuses, drains are partition-disjoint.
                                    mc = PPART // colsplit
                                    for c in range(colsplit):
                                        mi = nc.tensor.matmul(
                                            ps[g][mc * c:mc * (c + 1), :],
                                            lhsT=xt_sb[:, k,
                                                       bt * PPART + mc * c:
                                                       bt * PPART + mc * (c + 1)],
                                            rhs=rhs,
                                            start=start, stop=stop,
                                            tile_position=(0, mc * c),
                                        )
                                        mm_meta[mi.ins.name] = (
                                            _rep, lg, bt, k, (g, c), "main")
                        if not do_y:
                            pass
                        elif ytile:
                            for g in range(G):
                                mi = nc.tensor.matmul(
                                    ps[g][:],
                                    lhsT=yp_sb[32 * g:32 * (g + 1), lg,
                                               bt * PPART:(bt + 1) * PPART],
                                    rhs=wp_sb[32 * g:32 * (g + 1), lg, :],
                                    start=False, stop=True,
                                    tile_position=(32 * g, 0),
                                )
                                mm_meta[mi.ins.name] = (_rep, lg, bt, None, g,
                                                        "y")
                        else:
                            ya_lhs = yp_sb[:, bt * PPART:(bt + 1) * PPART]
                            for g in range(G):
                                nc.tensor.matmul(
                                    ps[g][:], lhsT=ya_lhs,
                                    rhs=wp_sb[:, lg * G + g, :],
                                    start=False, stop=True,
                                )
                        for g in range(G):
                            head = lg * G + g
                            kl = int(k_pos[head])
                            col = bt * L + head
                            if stage2 == "none":
                                continue
                            if rowsplit:
                                zt = scratch_pool.tile([PPART, H], f32,
                                                       tag="zt")
                                nc.vector.tensor_tensor(
                                    zt[:], ps[g][:], psb[g][:],
                                    mybir.AluOpType.add,
                                )
                                sc = scratch_pool.tile([PPART, H], bf16,
                                                       tag="sc")
                                if kl > 0:
                                    nc.scalar.activation(
                                        sc[:, :kl], zt[:, :kl],
                                        mybir.ActivationFunctionType.Relu,
                                        accum_out=pos_sb[:, col:col + 1],
                                    )
                                if kl < H:
                                    nc.scalar.activation(
                                        sc[:, kl:], zt[:, kl:],
                                        mybir.ActivationFunctionType.Relu,
                                        accum_out=neg_sb[:, col:col + 1],
                                    )
                                continue
                            if stage2 == "act2p":
                                # relu main-out written back to PSUM in
                                # place: faster ACT access path and no
                                # SBUF scratch writes contending with the
                                # PE's rhs streams
                                if kl > 0:
                                    nc.scalar.activation(
                                        ps[g][:, :kl], ps[g][:, :kl],
                                        mybir.ActivationFunctionType.Relu,
                                        accum_out=pos_sb[:, col:col + 1],
                                    )
                                if kl < H:
                                    nc.scalar.activation(
                                        ps[g][:, kl:], ps[g][:, kl:],
                                        mybir.ActivationFunctionType.Relu,
                                        accum_out=neg_sb[:, col:col + 1],
                                    )
                                continue
                            sc = scratch_pool.tile([PPART, H], bf16, tag="sc")
                            if kl > 0:
                                nc.scalar.activation(
                                    sc[:, :kl], ps[g][:, :kl],
                                    mybir.ActivationFunctionType.Relu,
                                    accum_out=pos_sb[:, col:col + 1],
                                )
                            if kl < H:
                                if stage2 == "ttr":
                                    # negative-sign range reduced on DVE
                                    # (tensor_tensor_reduce, as in the QR
                                    # kernel) to halve the ScalarE load:
                                    # accum = sum(max(z, 0))
                                    scd = scratch_pool.tile(
                                        [PPART, H], bf16, tag="scd")
                                    nc.vector.tensor_tensor_reduce(
                                        scd[:, kl:], ps[g][:, kl:],
                                        zero_sb[:, :H - kl], 1.0, 0.0,
                                        mybir.AluOpType.max,
                                        mybir.AluOpType.add,
                                        accum_out=neg_sb[:, col:col + 1],
                                    )
                                elif stage2 == "split":
                                    # negative-sign range reduced on DVE to
                                    # halve the ScalarE load
                                    nc.vector.tensor_scalar(
                                        sc[:, kl:], ps[g][:, kl:],
                                        0.0, None, mybir.AluOpType.max,
                                        op1=mybir.AluOpType.add,
                                        accum_out=neg_sb[:, col:col + 1],
                                    )
                                else:
                                    nc.scalar.activation(
                                        sc[:, kl:], ps[g][:, kl:],
                                        mybir.ActivationFunctionType.Relu,
                                        accum_out=neg_sb[:, col:col + 1],
                                    )

            # --- epilogue: out = pos - neg + b2 ---
            for bt in range(n_bt):
                o = out_pool.tile([PPART, L], f32, tag="o")
                nc.vector.tensor_tensor(
                    o[:], pos_sb[:, bt * L:(bt + 1) * L],
                    neg_sb[:, bt * L:(bt + 1) * L], mybir.AluOpType.subtract,
                )
                nc.vector.tensor_tensor(
                    o[:], o[:], b2_sb[:], mybir.AluOpType.add,
                )
                nc.sync.dma_start(out_d.ap()[bt * PPART:(bt + 1) * PPART, :], o[:])

    if rowsplit:
        assert ytile and do_y and colsplit == 1 and mm_mode == "normal"
    if reorder == "y":
        assert ytile and do_y
        ydedup_pe(nc, mm_meta)
    elif reorder:
        assert ytile and do_y and not resident
        reorder_dedup_pe(nc, mm_meta)
    split_multi_waits(nc)
    return nc


def prep_host(x, y, Wx, Wp, b1, W2, b2, parent_idx, n_bt: int = 8,
              ytile: bool = True):
    """Host-side data prep.  Returns (in_maps per core, k_pos)."""
    x = np.asarray(x, np.float32)
    y = np.asarray(y, np.float32)
    Wx = np.asarray(Wx, np.float32)
    Wp = np.asarray(Wp, np.float32)
    b1 = np.asarray(b1, np.float32)
    W2 = np.asarray(W2, np.float32)
    b2 = np.asarray(b2, np.float32)
    parent_idx = np.asarray(parent_idx)
    NP = parent_idx.shape[1]

    bc = n_bt * PPART

    # |W2| folding + sign-partition permutation of the H axis (per head)
    s = np.abs(W2)                       # [L, H]
    k_pos = np.zeros(L, np.int64)
    perm = np.zeros((L, H), np.int64)
    for l in range(L):
        posm = W2[l] >= 0
        p_idx = np.concatenate([np.where(posm)[0], np.where(~posm)[0]])
        perm[l] = p_idx
        k_pos[l] = int(posm.sum())

    # wxp[lg, kt, g, p, h'] = Wx[l, kt*128+p, perm[l,h']] * s[l, perm[l,h']]
    wxp = np.empty((N_LG, KT, G, PPART, H), BF16)
    for l in range(L):
        m = (Wx[l] * s[l][None, :])[:, perm[l]]          # [D, H]
        wxp[l // G, :, l % G] = m.reshape(KT, PPART, H).astype(BF16)

    b2r = np.broadcast_to(b2[None, :], (PPART, L)).astype(np.float32).copy()

    if ytile:
        # wp[lg, 32g+j] = parent-j weights of head l=lg*G+g (scaled/permuted);
        # row 32g+NP = b1 row (pairs with the ones row of yp).
        wp = np.zeros((N_LG, PPART, H), np.float32)
        for l in range(L):
            lg, g = l // G, l % G
            for j in range(NP):
                wp[lg, 32 * g + j] = (Wp[l, j] * s[l])[perm[l]]
            wp[lg, 32 * g + NP] = (b1[l] * s[l])[perm[l]]
        wp = wp.astype(BF16)
    else:
        # dense label-indexed parent weights + b1 via ones row (K=128 padded)
        Wfull = np.zeros((L, L, H), np.float32)
        for l in range(L):
            for p in range(NP):
                Wfull[l, parent_idx[l, p]] += Wp[l, p]
        wpa = np.zeros((PPART, L, H), np.float32)
        for l in range(L):
            wpa[:L, l, :] = (Wfull[l] * s[l][None, :])[:, perm[l]]
            wpa[L, l, :] = (b1[l] * s[l])[perm[l]]
        wpa = wpa.astype(BF16)

    in_maps = []
    for c in range(N_CORES):
        xs = x[c * bc:(c + 1) * bc]                       # [bc, D]
        ys = y[c * bc:(c + 1) * bc]                       # [bc, L]
        xT = np.ascontiguousarray(xs.T).astype(BF16)      # [D, bc]
        m = {"xT": xT, "wxp": wxp, "b2r": b2r}
        if ytile:
            yp = np.zeros((N_LG, PPART, bc), np.float32)
            for l in range(L):
                lg, g = l // G, l % G
                for j in range(NP):
                    yp[lg, 32 * g + j] = ys[:, parent_idx[l, j]]
                yp[lg, 32 * g + NP] = 1.0
            m["yp"] = yp.astype(BF16)
            m["wp"] = wp
        else:
            ya = np.zeros((PPART, bc), np.float32)
            ya[:L] = ys.T
            ya[L] = 1.0
            m["ya"] = ya.astype(BF16)
            m["wpa"] = wpa
        in_maps.append(m)
    return in_maps, k_pos


def reference_host(x, y, Wx, Wp, b1, W2, b2, parent_idx):
    """numpy fp32 reference (for sim tests inside this module)."""
    base = np.einsum('bd,ldh->blh', x, Wx)
    parents = y[:, parent_idx]
    pc = np.einsum('blp,lph->blh', parents, Wp)
    h = np.maximum(base + pc + b1[None], 0.0)
    return np.einsum('blh,lh->bl', h, W2) + b2


_CACHE = {}


def kernel(x, y, Wx, Wp, b1, W2, b2, parent_idx):
    from concourse.bass_utils import run_bass_kernel_spmd

    x = np.asarray(x)
    n_bt = x.shape[0] // N_CORES // PPART
    in_maps, k_pos = prep_host(x, y, Wx, Wp, b1, W2, b2, parent_idx, n_bt=n_bt)

    key = (n_bt, True, tuple(int(v) for v in k_pos))
    if key not in _CACHE:
        _CACHE[key] = build_program(n_bt, k_pos)
    nc = _CACHE[key]

    res = run_bass_kernel_spmd(nc, in_maps, core_ids=list(range(N_CORES)))
    out = np.concatenate([res.results[c]["out"] for c in range(N_CORES)], axis=0)
    return out.astype(np.float32)



# revision 34
# speedup vs baseline: 1.1035x; 1.0499x over previous
"""Trainium2 Bass kernel for nn_AutoregressiveHead.

Reference computation (L=32 heads, D=1024, H=512, B=8192, P=2 parents):
    base = einsum('bd,ldh->blh', x, Wx)
    parents = y[:, parent_idx]                  # [B, L, P]
    pc = einsum('blp,lph->blh', parents, Wp)
    h = relu(base + pc + b1)
    out = einsum('blh,lh->bl', h, W2) + b2      # [B, L]

Strategy (data-parallel over B across 8 NeuronCores, weights replicated):
  * Per core: B_core=1024 batch rows.  The dominant compute is the 32
    per-head matmuls x @ Wx[l] ([1024,1024]@[1024,512], bf16, fp32 PSUM).
  * The parent gather + b1 are folded into one extra K<=128 matmul:
    y_aug = [y^T; ones; 0-pad] (K=128 padded), W_aug[l] = [Wp scattered to
    dense label rows; b1[l]; 0] so  x@Wx[l] + y_aug@W_aug[l] = base+pc+b1.
  * |W2[l,h]| is folded into Wx/W_aug columns on the host, and columns are
    permuted so positive-sign-W2 columns come first.  Then
        out[b,l] = sum_pos relu(z') - sum_neg relu(z') + b2[l]
    which the Scalar engine computes with activation(Relu, accum_out=...)
    over the two column ranges -- no second matmul stage, h never stored.
  * PSUM tile [128, 512] per head; l processed in groups of 4 with the Wx
    slab for the group resident in SBUF (triple-buffered across groups).

Measured on HW (8 axon TRN2 cores): ~575 us exec (quiet terminal; the
interleaved equal-batch slope estimator in test.py; terminal load adds
up to ~20% session-level drift), rel err 0.0024.

Cost model (closed against measurement): every N=512 matmul "stream
pass" costs ~258 ns = 512 columns at an effective ~2.0 GHz (the P0
sustained-load downclock), regardless of structure.  Invariant under:
LDWEIGHTS dedup (runs-of-4 same-lhsT with redundant LDWs deleted
post-schedule, bit-exact), identical rhs/lhsT operands, start=True
non-accumulating mode, 2x/4x col-tiling via tile_position, single-DMA
weight slabs, and -- decisively -- an rhs bitcast to fp8 (same column
count, HALF the bytes: identical 572.9us vs 572.9us base), which rules
out per-xbus byte rate and pins the wall to column-streaming rate
alone.  Main GEMM = 2048 passes * 258ns = 531us + y-block 37us + ACT
leak 9us ~= 577us.  Only a pass-count reduction can beat this, and the
only mechanism (fp8 DoubleRow, K=256/pass, needs BOTH operands e4m3)
measures rel err 0.038 > the 2e-2 gate; passing hybrid fractions save
<5% while eating the correctness margin -- rejected.

Tried and rejected (this + prior session A/B): fp8 DoubleRow
(numerics), K=64 row-split to 2 PSUM banks + DVE combine (this walrus
rejects any DVE instruction reading PSUM: birverifier assertion), DVE
tensor_scalar+accum (device crash), tensor_tensor_reduce ("ISA wrong
length"), G=8 runs-of-8 lhsT (-20%), PSUM-in-place relu (-4.5% and
re-measured +4us), y-strip LDW widening/dedup (bit-exact, 0 delta),
single-DMA weight slab (0 delta), fp8 activation scratch output
(walrus rejects), y-aux folding into main k-tiles (exact counting:
every head needs 1024 x-dims + 1..3 aux rows vs exactly 8*128
contraction rows -- even head 0 is one row over; a K=1 bias matmul
still costs a full 512-column pass, so the ones-row is irreducible),
sem-increment merging (refuted by cs4: 4x increments, 0 delta).
Estimator bias audit: per-call dispatch overhead is ~5.9ms for BOTH
repeats=5 and repeats=17 programs, so the slope is size-unbiased.
Post-schedule instruction-stream passes that MOVE sem-carrying
instructions deadlock the device (Tile wait counts are baked against
the scheduled order); delete-only/in-place-mutate passes are safe.
"""

import os
import numpy as np
import ml_dtypes

import bass_rust
import concourse.bass as bass
import concourse.tile as tile
from concourse import mybir
from concourse.vector_clock import ScopedClock

BF16 = ml_dtypes.bfloat16

N_CORES = 8
B, D, H, L = 8192, 1024, 512, 32
B_CORE = B // N_CORES          # 1024
PPART = 128                    # partition size
KT = D // PPART                # 8 k-tiles over D
G = 4                          # heads per PSUM group
N_LG = L // G                  # 8 groups


class SplitDrainTileContext(tile.TileContext):
    """The walrus build in this container rejects >1 sem waits on the tail
    Drain ("Too many sync wait commands").  Redistribute the global-clock
    waits onto single-wait nops preceding the drain."""

    def _drain_and_barrier(self, tick_clock, wait_clock):
        probe = self.nc.sync.nop(nofuse=True)
        wait_clock.add_sem_waits(
            probe.ins, ScopedClock({None: tick_clock.global_clock})
        )
        si = probe.ins.sync_info
        waits = list(si.on_wait) if si is not None and si.on_wait else []
        if len(waits) > 1:
            si.on_wait = waits[:1]
            for w in waits[1:]:
                n = self.nc.sync.nop(nofuse=True)
                n.ins.sync_info = bass_rust.SyncInfo(on_wait=[w], on_update=[])
        self.nc.sync.drain()
        self.nc.all_engine_barrier()
        assert self.sems is not None
        popped = self.nc._tile_sem_poison_stack.pop()
        assert popped is self._sem_poison
        self.nc.clear_and_free_semaphores(list(self.sems.allocated().values()))
        self.nc.all_engine_barrier()


def split_multi_waits(nc, max_waits: int = 1):
    """This container's walrus rejects instructions carrying more than one
    sem-wait ("Too many sync wait commands").  Hoist extra waits onto
    single-wait NoOps inserted just before the instruction on the same
    engine (engine order is preserved; sems are monotonic, so waiting
    earlier on the same engine is equivalent)."""
    uid = 0
    for f in nc.m.functions:
        for blk in f.blocks:
            insts = blk.instructions
            new = []
            for inst in insts:
                si = inst.sync_info
                waits = list(si.on_wait) if si is not None and si.on_wait else []
                if len(waits) > max_waits:
                    for w in waits[:-max_waits]:
                        nop = mybir.InstNoOp(
                            name=f"splitw-{uid}", engine=inst.engine,
                            ins=[], outs=[],
                        )
                        uid += 1
                        nop.sync_info = bass_rust.SyncInfo(
                            on_wait=[w], on_update=[]
                        )
                        nc.register_instruction(nop, overwrite=True)
                        new.append(nop)
                    si.on_wait = waits[-max_waits:]
                new.append(inst)
            insts[:] = new


def reorder_dedup_pe(nc, mm_meta, n_kt=KT, n_g=G):
    """Rebuild the scheduled PE LDW/MM stream into k-outer/g-inner runs and
    drop the redundant per-MM LDWEIGHTS inside each same-lhsT run.

    The Tile scheduler emits one InstLdweights per InstMatmult (FIFO-paired,
    identical APs -- verified).  Within one (rep, lg, bt) iteration the four
    heads' matmuls at a given k-tile share lhsT = xT[k, bt]; loading it once
    and issuing the 4 matmuls back-to-back removes 3/4 of the weight-load
    traffic on the PE weight port, which is serialized with the rhs
    streaming on this hardware (measured 261 ns/mm at N=512 vs the 213 ns
    streaming floor).

    Per-PSUM-bank instruction order (start -> accumulate -> stop) is
    preserved: bank g still sees its k=0..7 matmuls in order followed by its
    y-strip matmul.  Sem waits are stall-only, so hoisting a dropped LDW's
    waits onto a nop placed just before the surviving LDW is equivalent.
    """
    import concourse.mybir as mybir_m

    blk = max(nc.m.functions[0].blocks, key=lambda b: len(b.instructions))
    insts = blk.instructions

    # FIFO-pair LDW -> MM on the PE stream
    from collections import deque
    q = deque()
    pairs = []      # (ldw, mm)
    for inst in insts:
        if getattr(inst, "engine", None) != mybir_m.EngineType.PE:
            continue
        if isinstance(inst, mybir_m.InstLdweights):
            q.append(inst)
        elif isinstance(inst, mybir_m.InstMatmult):
            pairs.append((q.popleft(), inst))
    assert not q, f"unpaired LDWEIGHTS: {len(q)}"

    groups = {}
    paired_ids = set()
    for ldw, mm in pairs:
        meta = mm_meta.get(mm.name)
        assert meta is not None, f"unknown matmul {mm.name}"
        rep, lg, bt, k, g, role = meta
        e = groups.setdefault((rep, lg, bt),
                              {"main": {}, "y": []})
        if role == "main":
            ent = e["main"].setdefault(k, {"ldws": [], "mms": {}})
            ent["ldws"].append(ldw)
            ent["mms"][g] = mm
        else:
            e["y"].append((g, ldw, mm))
        paired_ids.add(id(ldw))
        paired_ids.add(id(mm))

    uid = 0

    def wait_nops(waits):
        nonlocal uid
        out = []
        for w in waits:
            nop = mybir_m.InstNoOp(
                name=f"roww-{uid}", engine=mybir_m.EngineType.PE,
                ins=[], outs=[],
            )
            uid += 1
            nop.sync_info = bass_rust.SyncInfo(on_wait=[w], on_update=[])
            nc.register_instruction(nop, overwrite=True)
            out.append(nop)
        return out

    seq = []
    for key in sorted(groups):
        e = groups[key]
        for k in sorted(e["main"]):
            ent = e["main"][k]
            assert len(ent["ldws"]) == n_g and len(ent["mms"]) == n_g, (
                f"group {key} k={k}: {len(ent['ldws'])} ldws "
                f"{len(ent['mms'])} mms"
            )
            kept = ent["ldws"][0]
            extra_waits, extra_updates = [], []
            for d in ent["ldws"][1:]:
                si = d.sync_info
                if si is not None:
                    extra_waits += list(si.on_wait or [])
                    extra_updates += list(si.on_update or [])
            seq += wait_nops(extra_waits)
            if extra_updates:
                si = kept.sync_info
                if si is None:
                    kept.sync_info = bass_rust.SyncInfo(
                        on_wait=[], on_update=list(extra_updates))
                else:
                    si.on_update = list(si.on_update or []) + extra_updates
            seq.append(kept)
            for g in sorted(ent["mms"]):
                seq.append(ent["mms"][g])
        for g, ldw, mm in sorted(e["y"], key=lambda t: t[0]):
            seq += [ldw, mm]

    # splice: drop paired instructions, insert rebuilt sequence at the
    # position of the first dropped one
    new = []
    inserted = False
    for inst in insts:
        if id(inst) in paired_ids:
            if not inserted:
                new.extend(seq)
                inserted = True
            continue
        new.append(inst)
    assert inserted
    insts[:] = new


def ydedup_pe(nc, mm_meta):
    """Collapse each (lg, bt) group's four serial per-strip y-LDWEIGHTS
    (~107ns each on the PE weight port) into one 128-row load, by WIDENING
    the g0 strip LDW's access pattern in place and deleting the other three
    (they are sync-free, so deleting them changes no semaphore counts and
    no instruction moves).  The strip matmuls then contract against their
    32-row group of the one loaded array.

    A group is only rewritten when the scheduled stream is safe for it:
    g0's LDW precedes all four strip matmuls and no other LDWEIGHTS (which
    would clobber the array) sits between it and the last strip matmul.
    """
    import concourse.mybir as mybir_m
    from collections import deque

    blk = max(nc.m.functions[0].blocks, key=lambda b: len(b.instructions))
    insts = blk.instructions
    names = dict(mm_meta)

    strips = {}          # (rep, lg, bt) -> list[(g, ldw, mm)]
    q = deque()
    order = {}
    ldw_positions = []   # (pos, inst) of every LDW
    for pos, inst in enumerate(insts):
        if getattr(inst, "engine", None) != mybir_m.EngineType.PE:
            continue
        order[id(inst)] = pos
        if isinstance(inst, mybir_m.InstLdweights):
            q.append(inst)
            ldw_positions.append((pos, inst))
        elif isinstance(inst, mybir_m.InstMatmult):
            ldw = q.popleft()
            meta = names.get(inst.name)
            if meta is not None and meta[5] == "y":
                strips.setdefault(meta[:3], []).append((meta[4], ldw, inst))
    assert not q, f"unpaired LDWEIGHTS: {len(q)}"

    def sync_free(inst):
        si = inst.sync_info
        return si is None or (not si.on_wait and not si.on_update)

    drop = set()
    n_applied = n_skipped = 0
    for key, lst in strips.items():
        lst.sort(key=lambda t: t[0])
        if len(lst) != 4 or [g for g, _, _ in lst] != [0, 1, 2, 3]:
            n_skipped += 1
            continue
        ldw0 = lst[0][1]
        p0 = order[id(ldw0)]
        mm_pos = [order[id(mm)] for _, _, mm in lst]
        group_ldw_ids = {id(ldw) for _, ldw, _ in lst}
        if p0 >= min(mm_pos):
            n_skipped += 1
            continue
        if not all(sync_free(ldw) for _, ldw, _ in lst[1:]):
            n_skipped += 1
            continue
        # no foreign LDW may sit inside (p0, max(mm_pos))
        hi = max(mm_pos)
        foreign = any(p0 < pos < hi and id(ld) not in group_ldw_ids
                      for pos, ld in ldw_positions)
        if foreign:
            n_skipped += 1
            continue
        pap = ldw0.ins[0]
        ap = [list(d) for d in pap.ap]
        assert ap[0][1] == 32, f"unexpected strip LDW ap {ap}"
        ap[0][1] = 128
        pap.ap = ap
        ldw0.tile_size = (128, ldw0.tile_size[1])
        for _, ldw, _ in lst[1:]:
            drop.add(id(ldw))
        n_applied += 1

    if drop:
        insts[:] = [i for i in insts if id(i) not in drop]
    nc._ydedup_stats = (n_applied, n_skipped)


def build_program(n_bt: int, k_pos, repeats: int = 1, ytile: bool = True,
                  stage2: str = "act2", do_y: bool = True,
                  resident: bool = False, n_lg: int = N_LG,
                  reorder: bool = False, mm_mode: str = "normal",
                  colsplit: int = 1, rowsplit: bool = False,
                  onedma: bool = False):
    """Build the per-core Bass program.

    n_bt: number of 128-row batch tiles per core (8 for the real problem).
    k_pos: per-head split point -- columns [0:k_pos[l]) carry W2>=0,
           [k_pos[l]:H) carry W2<0 (after the host-side permutation).
    repeats: re-emit the whole compute `repeats` times (timing builds).
    ytile: pack the 4 per-head parent/bias matmuls of a group into
           concurrent 32-row-strip matmuls via tile_position (the strips
           execute in parallel in the PE array).
    """
    f32 = mybir.dt.float32
    bf16 = mybir.dt.bfloat16
    bc = n_bt * PPART

    nc = bass.Bass("TRN2", target_bir_lowering=False, debug=False,
                   num_devices=N_CORES)

    xT_d = nc.dram_tensor("xT", [D, bc], bf16, kind="ExternalInput")
    # wxp layout groups G heads contiguously per (lg, k) for 512KB DMAs
    wxp_d = nc.dram_tensor("wxp", [N_LG, KT, G, PPART, H], bf16,
                           kind="ExternalInput")
    b2r_d = nc.dram_tensor("b2r", [PPART, L], f32, kind="ExternalInput")
    if ytile:
        # per (lg): strip g holds [y_par0; y_par1; ones; 0...] for head lg*G+g
        yp_d = nc.dram_tensor("yp", [N_LG, PPART, bc], bf16,
                              kind="ExternalInput")
        wp_d = nc.dram_tensor("wp", [N_LG, PPART, H], bf16,
                              kind="ExternalInput")
    else:
        yp_d = nc.dram_tensor("ya", [PPART, bc], bf16, kind="ExternalInput")
        wp_d = nc.dram_tensor("wpa", [PPART, L, H], bf16,
                              kind="ExternalInput")
    out_d = nc.dram_tensor("out", [bc, L], f32, kind="ExternalOutput")

    with SplitDrainTileContext(nc) as tc:
        with (
            tc.tile_pool(name="const", bufs=1) as const_pool,
            tc.tile_pool(name="wx", bufs=3) as wx_pool,
            tc.tile_pool(name="psum", bufs=4 if rowsplit else 8,
                         space="PSUM") as psum_pool,
            tc.tile_pool(name="scratch", bufs=4) as scratch_pool,
            tc.tile_pool(name="outp", bufs=4) as out_pool,
        ):
            # --- resident tensors ---
            xt_sb = const_pool.tile([PPART, KT, bc], bf16, tag="xt")
            nc.sync.dma_start(
                xt_sb[:], xT_d.ap().rearrange("(kt p) b -> p kt b", p=PPART)
            )
            if ytile:
                yp_sb = const_pool.tile([PPART, N_LG, bc], bf16, tag="yp")
                nc.sync.dma_start(
                    yp_sb[:], yp_d.ap().rearrange("lg p b -> p lg b")
                )
                wp_sb = const_pool.tile([PPART, N_LG, H], bf16, tag="wp")
                nc.sync.dma_start(
                    wp_sb[:], wp_d.ap().rearrange("lg p h -> p lg h")
                )
            else:
                yp_sb = const_pool.tile([PPART, bc], bf16, tag="yp")
                nc.sync.dma_start(yp_sb[:], yp_d.ap())
                wp_sb = const_pool.tile([PPART, L, H], bf16, tag="wp")
                nc.sync.dma_start(wp_sb[:], wp_d.ap())
            b2_sb = const_pool.tile([PPART, L], f32, tag="b2")
            nc.sync.dma_start(b2_sb[:], b2r_d.ap())

            pos_sb = const_pool.tile([PPART, n_bt * L], f32, tag="pos")
            neg_sb = const_pool.tile([PPART, n_bt * L], f32, tag="neg")
            nc.vector.memset(pos_sb[:], 0.0)
            nc.vector.memset(neg_sb[:], 0.0)
            zero_sb = const_pool.tile([PPART, H], bf16, tag="zero")
            nc.vector.memset(zero_sb[:], 0.0)

            if resident:
                # diagnostic: all wx weights resident in SBUF (no DMA in the
                # repeat loop).  Only feasible for n_lg <= 4 (SBUF capacity).
                wxr_sb = const_pool.tile([PPART, n_lg, KT, G, H], bf16,
                                         tag="wxr")
                for lg in range(n_lg):
                    for k in range(KT):
                        nc.sync.dma_start(
                            wxr_sb[:, lg, k, :, :],
                            wxp_d.ap()[lg, k].rearrange("g p h -> p g h"),
                        )

            mm_meta = {}
            for _rep in range(repeats):
                for lg in range(n_lg):
                    if resident:
                        wx_sb = wxr_sb[:, lg]
                    elif onedma:
                        wx_sb = wx_pool.tile([PPART, KT, G, H], bf16, tag="wx")
                        nc.sync.dma_start(
                            wx_sb[:],
                            wxp_d.ap()[lg].rearrange("kt g p h -> p kt g h"),
                        )
                    else:
                        wx_sb = wx_pool.tile([PPART, KT, G, H], bf16, tag="wx")
                        for k in range(KT):
                            nc.sync.dma_start(
                                wx_sb[:, k, :, :],
                                wxp_d.ap()[lg, k].rearrange("g p h -> p g h"),
                            )
                    for bt in range(n_bt):
                        ps = [
                            psum_pool.tile([PPART, H], f32, tag="ps", name="ps")
                            for _ in range(G)
                        ]
                        if rowsplit:
                            # second bank per head: K rows 64..127 accumulate
                            # here; combined with ps[g] on DVE before stage 2.
                            psb = [
                                psum_pool.tile([PPART, H], f32, tag="psb",
                                               name="psb")
                                for _ in range(G)
                            ]
                        # g-outer / k-inner: lhsT changes every matmul, which
                        # measures ~1.5x faster per-mm than same-lhsT runs
                        # (PE weight-load pipelining quirk), and lets the
                        # ACT drain of ps[g] start before the bt finishes.
                        for g in range(G):
                            for k in range(KT):
                                lhsT = xt_sb[:, k,
                                             bt * PPART:(bt + 1) * PPART]
                                rhs = wx_sb[:, k, g, :]
                                start = (k == 0)
                                stop = (not do_y and k == KT - 1)
                                # mm_mode: timing diagnostics (results wrong
                                # except "normal")
                                if mm_mode in ("samerhs", "sameboth"):
                                    rhs = wx_sb[:, 0, 0, :]
                                if mm_mode in ("samelhs", "sameboth"):
                                    lhsT = xt_sb[:, 0, 0:PPART]
                                if mm_mode == "rhs8":
                                    # timing probe: same column count, half
                                    # the rhs bytes (values are garbage)
                                    rhs = wx_sb[:, k, g, :].bitcast(
                                        mybir.dt.float8e4)[:, :H]
                                if mm_mode == "allstart":
                                    start = True
                                    stop = True
                                if rowsplit:
                                    # two K=64 strips to different banks:
                                    # streams run concurrently on disjoint
                                    # lane groups, drains to separate banks
                                    for r in range(2):
                                        tgt = ps[g] if r == 0 else psb[g]
                                        mi = nc.tensor.matmul(
                                            tgt[:],
                                            lhsT=xt_sb[64 * r:64 * (r + 1), k,
                                                       bt * PPART:
                                                       (bt + 1) * PPART],
                                            rhs=wx_sb[64 * r:64 * (r + 1),
                                                      k, g, :],
                                            start=start,
                                            stop=(r == 1 and k == KT - 1),
                                        )
                                        mm_meta[mi.ins.name] = (
                                            _rep, lg, bt, k, (g, r), "main")
                                elif colsplit == 1:
                                    mi = nc.tensor.matmul(
                                        ps[g][:], lhsT=lhsT, rhs=rhs,
                                        start=start, stop=stop,
                                        skip_group_check=(mm_mode == "allstart"),
                                    )
                                    mm_meta[mi.ins.name] = (_rep, lg, bt, k,
                                                            g, "main")
                                else:
                                    # col-tiled: M split into `colsplit`
                                    # strips at tile_position (0, 64c/32c);
                                    # strips stream concurrently on separate
                                    # xbuses, drains are partition-disjoint.
                                    mc = PPART // colsplit
                                    for c in range(colsplit):
                                        mi = nc.tensor.matmul(
                                            ps[g][mc * c:mc * (c + 1), :],
                                            lhsT=xt_sb[:, k,
                                                       bt * PPART + mc * c:
                                                       bt * PPART + mc * (c + 1)],
                                            rhs=rhs,
                                            start=start, stop=stop,
                                            tile_position=(0, mc * c),
                                        )
                                        mm_meta[mi.ins.name] = (
                                            _rep, lg, bt, k, (g, c), "main")
                        if not do_y:
                            pass
                        elif ytile:
                            for g in range(G):
                                mi = nc.tensor.matmul(
                                    ps[g][:],
                                    lhsT=yp_sb[32 * g:32 * (g + 1), lg,
                                               bt * PPART:(bt + 1) * PPART],
                                    rhs=wp_sb[32 * g:32 * (g + 1), lg, :],
                                    start=False, stop=True,
                                    tile_position=(32 * g, 0),
                                )
                                mm_meta[mi.ins.name] = (_rep, lg, bt, None, g,
                                                        "y")
                        else:
                            ya_lhs = yp_sb[:, bt * PPART:(bt + 1) * PPART]
                            for g in range(G):
                                nc.tensor.matmul(
                                    ps[g][:], lhsT=ya_lhs,
                                    rhs=wp_sb[:, lg * G + g, :],
                                    start=False, stop=True,
                                )
                        for g in range(G):
                            head = lg * G + g
                            kl = int(k_pos[head])
                            col = bt * L + head
                            if stage2 == "none":
                                continue
                            if rowsplit:
                                zt = scratch_pool.tile([PPART, H], f32,
                                                       tag="zt")
                                nc.vector.tensor_tensor(
                                    zt[:], ps[g][:], psb[g][:],
                                    mybir.AluOpType.add,
                                )
                                sc = scratch_pool.tile([PPART, H], bf16,
                                                       tag="sc")
                                if kl > 0:
                                    nc.scalar.activation(
                                        sc[:, :kl], zt[:, :kl],
                                        mybir.ActivationFunctionType.Relu,
                                        accum_out=pos_sb[:, col:col + 1],
                                    )
                                if kl < H:
                                    nc.scalar.activation(
                                        sc[:, kl:], zt[:, kl:],
                                        mybir.ActivationFunctionType.Relu,
                                        accum_out=neg_sb[:, col:col + 1],
                                    )
                                continue
                            if stage2 == "act2p":
                                # relu main-out written back to PSUM in
                                # place: faster ACT access path and no
                                # SBUF scratch writes contending with the
                                # PE's rhs streams
                                if kl > 0:
                                    nc.scalar.activation(
                                        ps[g][:, :kl], ps[g][:, :kl],
                                        mybir.ActivationFunctionType.Relu,
                                        accum_out=pos_sb[:, col:col + 1],
                                    )
                                if kl < H:
                                    nc.scalar.activation(
                                        ps[g][:, kl:], ps[g][:, kl:],
                                        mybir.ActivationFunctionType.Relu,
                                        accum_out=neg_sb[:, col:col + 1],
                                    )
                                continue
                            sc = scratch_pool.tile([PPART, H], bf16, tag="sc")
                            if kl > 0:
                                nc.scalar.activation(
                                    sc[:, :kl], ps[g][:, :kl],
                                    mybir.ActivationFunctionType.Relu,
                                    accum_out=pos_sb[:, col:col + 1],
                                )
                            if kl < H:
                                if stage2 == "ttr":
                                    # negative-sign range reduced on DVE
                                    # (tensor_tensor_reduce, as in the QR
                                    # kernel) to halve the ScalarE load:
                                    # accum = sum(max(z, 0))
                                    scd = scratch_pool.tile(
                                        [PPART, H], bf16, tag="scd")
                                    nc.vector.tensor_tensor_reduce(
                                        scd[:, kl:], ps[g][:, kl:],
                                        zero_sb[:, :H - kl], 1.0, 0.0,
                                        mybir.AluOpType.max,
                                        mybir.AluOpType.add,
                                        accum_out=neg_sb[:, col:col + 1],
                                    )
                                elif stage2 == "split":
                                    # negative-sign range reduced on DVE to
                                    # halve the ScalarE load
                                    nc.vector.tensor_scalar(
                                        sc[:, kl:], ps[g][:, kl:],
                                        0.0, None, mybir.AluOpType.max,
                                        op1=mybir.AluOpType.add,
                                        accum_out=neg_sb[:, col:col + 1],
                                    )
                                else:
                                    nc.scalar.activation(
                                        sc[:, kl:], ps[g][:, kl:],
                                        mybir.ActivationFunctionType.Relu,
                                        accum_out=neg_sb[:, col:col + 1],
                                    )

            # --- epilogue: out = pos - neg + b2 ---
            for bt in range(n_bt):
                o = out_pool.tile([PPART, L], f32, tag="o")
                nc.vector.tensor_tensor(
                    o[:], pos_sb[:, bt * L:(bt + 1) * L],
                    neg_sb[:, bt * L:(bt + 1) * L], mybir.AluOpType.subtract,
                )
                nc.vector.tensor_tensor(
                    o[:], o[:], b2_sb[:], mybir.AluOpType.add,
                )
                nc.sync.dma_start(out_d.ap()[bt * PPART:(bt + 1) * PPART, :], o[:])

    if rowsplit:
        assert ytile and do_y and colsplit == 1 and mm_mode == "normal"
    if reorder == "y":
        assert ytile and do_y
        ydedup_pe(nc, mm_meta)
    elif reorder:
        assert ytile and do_y and not resident
        reorder_dedup_pe(nc, mm_meta)
    split_multi_waits(nc)
    return nc


def prep_host(x, y, Wx, Wp, b1, W2, b2, parent_idx, n_bt: int = 8,
              ytile: bool = True):
    """Host-side data prep.  Returns (in_maps per core, k_pos)."""
    x = np.asarray(x, np.float32)
    y = np.asarray(y, np.float32)
    Wx = np.asarray(Wx, np.float32)
    Wp = np.asarray(Wp, np.float32)
    b1 = np.asarray(b1, np.float32)
    W2 = np.asarray(W2, np.float32)
    b2 = np.asarray(b2, np.float32)
    parent_idx = np.asarray(parent_idx)
    NP = parent_idx.shape[1]

    bc = n_bt * PPART

    # |W2| folding + sign-partition permutation of the H axis (per head)
    s = np.abs(W2)                       # [L, H]
    k_pos = np.zeros(L, np.int64)
    perm = np.zeros((L, H), np.int64)
    for l in range(L):
        posm = W2[l] >= 0
        p_idx = np.concatenate([np.where(posm)[0], np.where(~posm)[0]])
        perm[l] = p_idx
        k_pos[l] = int(posm.sum())

    # wxp[lg, kt, g, p, h'] = Wx[l, kt*128+p, perm[l,h']] * s[l, perm[l,h']]
    wxp = np.empty((N_LG, KT, G, PPART, H), BF16)
    for l in range(L):
        m = (Wx[l] * s[l][None, :])[:, perm[l]]          # [D, H]
        wxp[l // G, :, l % G] = m.reshape(KT, PPART, H).astype(BF16)

    b2r = np.broadcast_to(b2[None, :], (PPART, L)).astype(np.float32).copy()

    if ytile:
        # wp[lg, 32g+j] = parent-j weights of head l=lg*G+g (scaled/permuted);
        # row 32g+NP = b1 row (pairs with the ones row of yp).
        wp = np.zeros((N_LG, PPART, H), np.float32)
        for l in range(L):
            lg, g = l // G, l % G
            for j in range(NP):
                wp[lg, 32 * g + j] = (Wp[l, j] * s[l])[perm[l]]
            wp[lg, 32 * g + NP] = (b1[l] * s[l])[perm[l]]
        wp = wp.astype(BF16)
    else:
        # dense label-indexed parent weights + b1 via ones row (K=128 padded)
        Wfull = np.zeros((L, L, H), np.float32)
        for l in range(L):
            for p in range(NP):
                Wfull[l, parent_idx[l, p]] += Wp[l, p]
        wpa = np.zeros((PPART, L, H), np.float32)
        for l in range(L):
            wpa[:L, l, :] = (Wfull[l] * s[l][None, :])[:, perm[l]]
            wpa[L, l, :] = (b1[l] * s[l])[perm[l]]
        wpa = wpa.astype(BF16)

    in_maps = []
    for c in range(N_CORES):
        xs = x[c * bc:(c + 1) * bc]                       # [bc, D]
        ys = y[c * bc:(c + 1) * bc]                       # [bc, L]
        xT = np.ascontiguousarray(xs.T).astype(BF16)      # [D, bc]
        m = {"xT": xT, "wxp": wxp, "b2r": b2r}
        if ytile:
            yp = np.zeros((N_LG, PPART, bc), np.float32)
            for l in range(L):
                lg, g = l // G, l % G
                for j in range(NP):
                    yp[lg, 32 * g + j] = ys[:, parent_idx[l, j]]
                yp[lg, 32 * g + NP] = 1.0
            m["yp"] = yp.astype(BF16)
            m["wp"] = wp
        else:
            ya = np.zeros((PPART, bc), np.float32)
            ya[:L] = ys.T
            ya[L] = 1.0
            m["ya"] = ya.astype(BF16)
            m["wpa"] = wpa
        in_maps.append(m)
    return in_maps, k_pos


def reference_host(x, y, Wx, Wp, b1, W2, b2, parent_idx):
    """numpy fp32 reference (for sim tests inside this module)."""
    base = np.einsum('bd,ldh->blh', x, Wx)
    parents = y[:, parent_idx]
    pc = np.einsum('blp,lph->blh', parents, Wp)
    h = np.maximum(base + pc + b1[None], 0.0)
    return np.einsum('blh,lh->bl', h, W2) + b2


_CACHE = {}


def kernel(x, y, Wx, Wp, b1, W2, b2, parent_idx):
    from concourse.bass_utils import run_bass_kernel_spmd

    x = np.asarray(x)
    n_bt = x.shape[0] // N_CORES // PPART
    in_maps, k_pos = prep_host(x, y, Wx, Wp, b1, W2, b2, parent_idx, n_bt=n_bt)

    key = (n_bt, True, tuple(int(v) for v in k_pos))
    if key not in _CACHE:
        _CACHE[key] = build_program(n_bt, k_pos)
    nc = _CACHE[key]

    res = run_bass_kernel_spmd(nc, in_maps, core_ids=list(range(N_CORES)))
    out = np.concatenate([res.results[c]["out"] for c in range(N_CORES)], axis=0)
    return out.astype(np.float32)

